# revision 9
# baseline (speedup 1.0000x reference)
"""Trainium2 Bass kernel for nn_NoBrainEncoderBlock_31662498906140.

out = softmax_n( clip( cos(q1_row, k1_row_n) * mask, 0, 1 ) )

Only q1, k1, mask affect the output (q2, k2, temp are unused by the math),
so only those are transferred to the device.

Design (208.8us f32 baseline -> ~160us):
  - k1 and q1n are cast to fp16 during host-side input prep in kernel()
    (alongside the host q1 normalization the baseline already did). This
    halves HBM traffic: measured per-core DMA floor drops from 194us
    (f32 at 346 GB/s, i.e. the ~358 GB/s HBM-per-NeuronCore limit) to
    86us (fp16, 2 MiB transfers). fp16 end-to-end rel err is 2e-5 vs
    the 2e-2 gate.
  - Per-core compute (measured in isolation): DVE runs the 64 fp16
    dot-reduces (scalar_tensor_tensor is 1x-mode at any dtype;
    2.21us/op = 141.5us) and ACT the 64 Square+accum ops (dtype-
    independent (N+352)/1.2 = 2.08us/op = 133.5us) plus the softmax
    tails. DVE is the critical chain; DMA hides fully underneath.
  - q1n arrives pre-replicated from the host as a [128, bpc*d] fp16
    block (one 2 MiB DMA) instead of a GPSIMD partition_broadcast
    chain, keeping Pool off the startup path.
  - Row tails are software-pipelined: rsqrt (exp(-0.5*ln x), one ACT
    table set) is emitted right after each row's last square; the
    DVE portion runs one row later (u==1) and the normalize+store two
    rows later (u==3), so the DVE reciprocal never waits on the GPSIMD
    partition all-reduce mid-stream.
  - All DMA goes through the SP HWDGE ring; sharding is data-parallel
    over batch B across the 8 cores (bpc=4 rows/core), natural
    [128 n-rows, d] tile layout, 8x 2 MiB double buffering.
"""

import numpy as np

B, N, D = 32, 2048, 2048
NCORES = 8
BPC = B // NCORES      # 4 batch rows per core
P = 128                # SBUF partitions
NT = N // P            # 16 n-columns per partition
BD = BPC * NT          # 64 stat columns per core
KBUFS = 8              # k1 tile buffer depth
VPAIR = 4              # n-rows per partition per DMA (2 MiB fp16 transfers)

_FNS = {}


def _build_tile_program(nc, tile, mybir, bass_isa, q1n, k1, mask, bpc, n, d, reps=1):
    """Per-core Tile program. q1n [bpc,d] fp16, k1 [bpc,n,d] fp16, mask [bpc,n] f32."""
    from contextlib import ExitStack, nullcontext

    f32 = mybir.dt.float32
    f16 = mybir.dt.float16
    p = 128
    nt = n // p
    bd = bpc * nt

    v = VPAIR
    nu = nt // v
    out = nc.dram_tensor("out", [bpc, n], f32, kind="ExternalOutput")

    # tile u: partition p holds rows n = p*nt + u*v .. +v-1 (v*d contiguous fp16)
    k1r = k1[:, :, :].rearrange("b (p u v) d -> b u p (v d)", p=p, u=nu, v=v)
    mask_r = mask[:, :].rearrange("b (p j) -> p b j", p=p)        # [128, bpc, nt]
    out_r = out[:, :].rearrange("b (p j) -> p b j", p=p)          # [128, bpc, nt]

    with tile.TileContext(nc) as tc, ExitStack() as ctx:
        stat = ctx.enter_context(tc.tile_pool(name="stat", bufs=1))
        kpool = ctx.enter_context(tc.tile_pool(name="kp", bufs=KBUFS))
        scr = ctx.enter_context(tc.tile_pool(name="scr", bufs=4))
        sqscr = ctx.enter_context(tc.tile_pool(name="sqscr", bufs=4))
        # q/mask get 2 buffers: with a single buffer, iteration i+1's qrep
        # DMA write-after-read hazards against iteration i's LAST dot
        # (which reads qb), serializing a ~7us DMA onto the DVE critical
        # path at every rep boundary. Two buffers let it prefetch.
        qpool = ctx.enter_context(tc.tile_pool(name="qp", bufs=2))
        mpool = ctx.enter_context(tc.tile_pool(name="mp", bufs=2))

        ctx.enter_context(tc.For_i(0, reps, 1) if reps > 1 else nullcontext())

        # q1n arrives pre-replicated from the host as [128, bpc*d] fp16 --
        # one 2 MiB DMA, no GPSIMD broadcast chain on the critical path.
        # Small transfers (q, mask, outputs) go through the ACT sequencer's
        # HWDGE ring (nc.scalar) so they never queue ahead of k1 loads on
        # the SP ring (nc.sync).
        qrep_t = qpool.tile([p, bpc * d], f16, tag="qrep")
        nc.sync.dma_start(out=qrep_t, in_=q1n[:, :])
        qb = [qrep_t[:, b * d : (b + 1) * d] for b in range(bpc)]

        # first k1 tile issues immediately on the SP ring
        kt0 = kpool.tile([p, v * d], f16, tag="kt")
        nc.sync.dma_start(out=kt0, in_=k1r[0, 0])

        mask_all = mpool.tile([p, bd], f32, tag="mask")
        nc.sync.dma_start(
            out=mask_all[:].rearrange("p (b j) -> p b j", b=bpc), in_=mask_r
        )

        dot_all = stat.tile([p, bd], f32, tag="dot")
        sq_all = stat.tile([p, bd], f32, tag="sq")
        rsq = stat.tile([p, bd], f32, tag="rsq")
        rsq2 = stat.tile([p, bd], f32, tag="rsq2")
        cosm = stat.tile([p, bd], f32, tag="cosm")
        s = stat.tile([p, bd], f32, tag="s")
        sc = stat.tile([p, bd], f32, tag="sc")
        e_all = stat.tile([p, bd], f32, tag="e")
        esum = stat.tile([p, bpc], f32, tag="esum")
        tot = stat.tile([p, bpc], f32, tag="tot")
        rtot = stat.tile([p, bpc], f32, tag="rtot")
        o_all = stat.tile([p, bd], f32, tag="o")

        def tail_rsqrt(b):
            # emitted right after row b's last square: rsqrt(x)=exp(-0.5*ln x)
            sl = slice(b * nt, (b + 1) * nt)
            nc.scalar.activation(
                out=rsq[:, sl], in_=sq_all[:, sl],
                func=mybir.ActivationFunctionType.Ln,
            )
            nc.scalar.activation(
                out=rsq2[:, sl], in_=rsq[:, sl],
                func=mybir.ActivationFunctionType.Exp, scale=-0.5,
            )

        def tail_a(b):
            # cos -> mask -> clip -> exp(+accum) -> partition all-reduce
            sl = slice(b * nt, (b + 1) * nt)
            nc.vector.tensor_mul(cosm[:, sl], dot_all[:, sl], rsq2[:, sl])
            # clip(cos*mask, 0, 1) == max(cos, 0)*mask: mask is in [0,1] and
            # |cos| <= 1, so the upper clip never binds; one fused STT op
            nc.vector.scalar_tensor_tensor(
                out=sc[:, sl], in0=cosm[:, sl], scalar=0.0, in1=mask_all[:, sl],
                op0=mybir.AluOpType.max, op1=mybir.AluOpType.mult,
            )
            nc.scalar.activation(
                out=e_all[:, sl], in_=sc[:, sl],
                func=mybir.ActivationFunctionType.Exp,
                accum_out=esum[:, b : b + 1],
            )
            nc.gpsimd.partition_all_reduce(
                tot[:, b : b + 1], esum[:, b : b + 1], p, bass_isa.ReduceOp.add
            )

        def tail_b(b):
            # normalize + store; scheduled two tile-blocks after tail_a so
            # the DVE reciprocal never waits on the GPSIMD all-reduce
            sl = slice(b * nt, (b + 1) * nt)
            nc.vector.reciprocal(rtot[:, b : b + 1], tot[:, b : b + 1])
            nc.vector.tensor_scalar_mul(
                o_all[:, sl], e_all[:, sl], rtot[:, b : b + 1]
            )
            nc.sync.dma_start(out=out_r[:, b, :], in_=o_all[:, sl])

        rotate = reps > 1
        for b in range(bpc):
            for u in range(nu):
                if b == 0 and u == 0:
                    kt = kt0
                else:
                    kt = kpool.tile([p, v * d], f16, tag="kt")
                    nc.sync.dma_start(out=kt, in_=k1r[b, u])
                for vv in range(v):
                    col = b * nt + u * v + vv
                    kslice = kt[:, vv * d : (vv + 1) * d]
                    dot_out = scr.tile([p, 1], f16, tag="dotout")
                    nc.vector.scalar_tensor_tensor(
                        out=dot_out.broadcast_to([p, d]),
                        in0=kslice,
                        scalar=1.0,
                        in1=qb[b],
                        op0=mybir.AluOpType.mult,
                        op1=mybir.AluOpType.mult,
                        accum_out=dot_all[:, col : col + 1],
                    )
                    sq_out = sqscr.tile([p, 1], f16, tag="sqout")
                    nc.scalar.activation(
                        out=sq_out.broadcast_to([p, d]),
                        in_=kslice,
                        func=mybir.ActivationFunctionType.Square,
                        accum_out=sq_all[:, col : col + 1],
                    )
                if u == 1 and (b > 0 or rotate):
                    # with rotate, row bpc-1's tail of the PREVIOUS loop
                    # iteration runs here (iteration 0 writes garbage into
                    # out[bpc-1]; every later iteration overwrites it with
                    # the correct, identical value)
                    tail_a(b - 1 if b > 0 else bpc - 1)
                if u == 3 and (b > 0 or rotate):
                    tail_b(b - 1 if b > 0 else bpc - 1)
            tail_rsqrt(b)
            if b == bpc - 1 and not rotate:
                tail_a(b)
                tail_b(b)

    return out


def _get_fn(reps=1):
    if reps in _FNS:
        return _FNS[reps]
    import jax
    from jax.sharding import Mesh, NamedSharding, PartitionSpec
    import concourse.tile as tile
    from concourse import bass_isa, mybir
    from concourse.bass2jax import bass_jit, bass_shard_map

    @bass_jit
    def shard_kernel(nc, q1n, k1, mask):
        return _build_tile_program(
            nc, tile, mybir, bass_isa, q1n, k1, mask, BPC, N, D, reps=reps
        )

    shard_kernel.__name__ = f"nbeb_{reps}"

    devices = jax.devices()[:NCORES]
    mesh = Mesh(np.array(devices), ("core",))
    pspec = PartitionSpec("core")
    fn = bass_shard_map(
        shard_kernel,
        mesh=mesh,
        in_specs=(pspec, pspec, pspec),
        out_specs=pspec,
    )
    _FNS[reps] = (fn, mesh, NamedSharding(mesh, pspec))
    return _FNS[reps]


def _prep_inputs(q1, k1, mask):
    q1 = np.asarray(q1, dtype=np.float32)
    nrm = np.linalg.norm(q1, axis=-1, keepdims=True)
    q1n = (q1 / np.maximum(nrm, 1e-12)).astype(np.float16)
    # replicate per-core q rows across the 128 partitions:
    # [B, D] -> [NCORES, 128, BPC*D] -> [NCORES*128, BPC*D]
    qrep = np.ascontiguousarray(
        np.broadcast_to(
            q1n.reshape(NCORES, 1, BPC * D), (NCORES, P, BPC * D)
        ).reshape(NCORES * P, BPC * D)
    )
    k1h = np.ascontiguousarray(np.asarray(k1)).astype(np.float16)
    mask = np.ascontiguousarray(np.asarray(mask, dtype=np.float32))
    return qrep, k1h, mask


def kernel(q1, k1, q2, k2, mask, temp):
    q1n, k1h, mask = _prep_inputs(q1, k1, mask)
    fn, _, _ = _get_fn()
    out = fn(q1n, k1h, mask)
    return np.asarray(out, dtype=np.float32).reshape(B, N)
